# revision 6
# baseline (speedup 1.0000x reference)
"""Trainium2 Bass kernel for nn_MiniAgentBlock (dense transformer block).

Sharding: DP=2 over batch x TP=4 within each batch (8 NeuronCores).
Core c: dp = c//4 (batch), tp = c%4 (4 q-heads / 1 kv-head, FF/4 slice).
All matmul phases run in transposed [feature, seq] layout with fp32r
matmuls (11-bit-mantissa inputs, fp32 accumulate). On-device AllReduce
after the attention output projection and ReduceScatter after the FFN
down projection, within each 4-core group. The residual x1 = x + attn is
folded into the ReduceScatter as 0.25*x1 per core, so the program is
identical on every core (pure SPMD, no core-dependent slicing).
"""
import sys
if "/opt/trn_rl_repo" not in sys.path:
    sys.path.insert(0, "/opt/trn_rl_repo")

import numpy as np
import concourse.bass as bass
import concourse.mybir as mybir
import concourse.tile as tile
from concourse import bacc
from concourse.bass_utils import run_bass_kernel_spmd

f32 = mybir.dt.float32
f32r = mybir.dt.float32r
AL = mybir.AluOpType
AF = mybir.ActivationFunctionType

B, S, H = 2, 2048, 2048
NH, NKV, HD = 16, 4, 128
FF = 5632
EPS = 1e-5
TPN = 4
QH = NH // TPN           # 4 q heads per core
FFS = FF // TPN          # 1408
FCT = FFS // 128         # 11 FF col tiles
SSL = S // TPN           # 512 output seq cols per core
NHT = H // 128           # 16 H tiles
NST = S // 128           # 16 seq tiles
NSB = S // 512           # 4 seq blocks
GROUPS = [[0, 1, 2, 3], [4, 5, 6, 7]]

# HD permutation: quadrant q: [evens 16q..16q+15 | odds 16q..16q+15]
PERM = np.zeros(HD, dtype=np.int64)
for _q in range(4):
    for _i in range(16):
        PERM[32 * _q + _i] = 2 * (16 * _q + _i)
        PERM[32 * _q + 16 + _i] = 2 * (16 * _q + _i) + 1
SHUF = [(i + 16) % 32 for i in range(32)]


def round_fp32r(a):
    u = np.ascontiguousarray(a, dtype=np.float32).view(np.uint32)
    low = u & np.uint32(0xFFF)
    keep = u >> np.uint32(12)
    round_up = (low > 0x800) | ((low == 0x800) & ((keep & 1) == 1))
    keep = keep + round_up.astype(np.uint32)
    return (keep << np.uint32(12)).view(np.float32)


def make_rope_tables(cos, sin, scale):
    C = np.zeros((HD, S), np.float32)
    S2 = np.zeros((HD, S), np.float32)
    for q in range(4):
        for i in range(16):
            pair = 16 * q + i
            C[32 * q + i] = cos[:, pair] * scale
            S2[32 * q + i] = -sin[:, pair] * scale
            C[32 * q + 16 + i] = cos[:, pair] * scale
            S2[32 * q + 16 + i] = sin[:, pair] * scale
    return C, S2


def _sb(x, sb):
    return x[:, sb * 512:(sb + 1) * 512]


def build():
    nc = bacc.Bacc("TRN2", target_bir_lowering=False, debug=False,
                   num_devices=8)

    def din(name, shape, dt=f32r):
        return nc.dram_tensor(name, list(shape), dt, kind="ExternalInput")

    xT = din("xT", [H, S], f32)
    wq = din("wq", [H, TPN * HD])          # permuted cols, fp32r-rounded
    wk = din("wk", [H, HD])                # permuted cols
    wv = din("wv", [H, HD])
    wo = din("wo", [QH * HD, H])
    wg = din("wg", [H, FFS])
    wu = din("wu", [H, FFS])
    wd = din("wd", [FFS, H])
    cq = din("cq", [HD, S], f32)           # cos/sqrt(HD) in permuted layout
    s2q = din("s2q", [HD, S], f32)
    ck = din("ck", [HD, S], f32)
    s2k = din("s2k", [HD, S], f32)
    wn1 = din("wn1", [128, NHT], f32)      # w_norm1[ht*128+p] at [p, ht]
    wn2 = din("wn2", [128, NHT], f32)
    tri = din("tri", [128, 128])           # f32r 0/1, tri[k,i] = (i >= k)
    ones = din("ones", [128, 1])           # f32r ones
    epsb = din("epsb", [128, 1], f32)      # EPS bias tile
    ident = din("ident", [128, 128], f32)  # f32 identity
    outsl = nc.dram_tensor("outsl", [H, SSL], f32, kind="ExternalOutput")

    with tile.TileContext(nc) as tc:
        with tc.tile_pool(name="pconst", bufs=1) as pconst, \
             tc.tile_pool(name="pdram", bufs=1, space="DRAM") as pdram:
            ones_t = pconst.tile([128, 1], f32r)
            tri_t = pconst.tile([128, 128], f32r)
            id_t = pconst.tile([128, 128], f32)
            wn1_t = pconst.tile([128, NHT], f32)
            wn2_t = pconst.tile([128, NHT], f32)
            eps_t = pconst.tile([128, 1], f32)
            nc.sync.dma_start(ones_t[:], ones[:])
            nc.sync.dma_start(tri_t[:], tri[:])
            nc.sync.dma_start(id_t[:], ident[:])
            nc.sync.dma_start(wn1_t[:], wn1[:])
            nc.sync.dma_start(wn2_t[:], wn2[:])
            nc.sync.dma_start(eps_t[:], epsb[:])

            outd = pdram.tile([QH, 128, S], f32r)
            ar_in = pdram.tile([H, S], f32)
            ar_out = pdram.tile([H, S], f32)
            x1d = pdram.tile([H, S], f32)
            mTd = pdram.tile([FCT, 128, S], f32r)
            rs_in = pdram.tile([NSB, H, 512], f32)
            rs_out = pdram.tile([H, 512], f32)

            with tc.tile_pool(name="phT", bufs=1) as phT:
                hT = phT.tile([128, NHT, S], f32r)

                # ---------- Phase A: rmsnorm1 -> hT ----------
                with tc.tile_pool(name="pA", bufs=1) as pA, \
                     tc.tile_pool(name="pAs", bufs=2) as pAs, \
                     tc.tile_pool(name="pAp", bufs=2, space="PSUM") as pAp:
                    for sb in range(NSB):
                        xsb = pA.tile([128, NHT, 512], f32, tag="xsb")
                        ss_ps = pAp.tile([1, 512], f32, tag="ss")
                        for ht in range(NHT):
                            nc.sync.dma_start(
                                xsb[:, ht, :],
                                _sb(xT[ht * 128:(ht + 1) * 128, :], sb))
                            sq = pAs.tile([128, 512], f32r, tag="sq")
                            nc.scalar.activation(sq[:], xsb[:, ht, :],
                                                 AF.Square)
                            nc.tensor.matmul(ss_ps[:], ones_t[:], sq[:],
                                             start=(ht == 0),
                                             stop=(ht == NHT - 1))
                        sd = pAs.tile([1, 512], f32, tag="sd")
                        nc.scalar.activation(sd[:], ss_ps[:], AF.Sqrt,
                                             bias=eps_t[0:1, :],
                                             scale=1.0 / H)
                        rr = pAs.tile([1, 512], f32, tag="rr")
                        nc.vector.reciprocal(rr[:], sd[:])
                        rb = pAs.tile([128, 512], f32, tag="rb")
                        nc.gpsimd.partition_broadcast(rb[:], rr[:])
                        for ht in range(NHT):
                            nc.vector.scalar_tensor_tensor(
                                out=_sb(hT[:, ht, :], sb),
                                in0=xsb[:, ht, :],
                                scalar=wn1_t[:, ht:ht + 1],
                                in1=rb[:], op0=AL.mult, op1=AL.mult)

                # ---------- Phase B: K/V projections + K rope ----------
                with tc.tile_pool(name="pkv", bufs=1) as pkv:
                    kT = pkv.tile([128, S], f32r)
                    v_nat = pkv.tile([128, NST, HD], f32r)

                    with tc.tile_pool(name="pB", bufs=1) as pB, \
                         tc.tile_pool(name="pBw", bufs=1) as pBw, \
                         tc.tile_pool(name="pBp", bufs=2,
                                      space="PSUM") as pBp:
                        wkt = pBw.tile([128, NHT, 128], f32r, tag="wB")
                        nc.sync.dma_start(
                            wkt[:], wk.rearrange("(o p) n -> p o n", p=128))
                        for sb in range(NSB):
                            ps = pBp.tile([128, 512], f32, tag="proj")
                            for ht in range(NHT):
                                nc.tensor.matmul(
                                    ps[:], wkt[:, ht, :],
                                    _sb(hT[:, ht, :], sb),
                                    start=(ht == 0), stop=(ht == NHT - 1))
                            ct_t = pB.tile([128, 512], f32, tag="ropeC", bufs=1)
                            st_t = pB.tile([128, 512], f32, tag="ropeS", bufs=1)
                            nc.sync.dma_start(ct_t[:], _sb(ck, sb))
                            nc.sync.dma_start(st_t[:], _sb(s2k, sb))
                            qs = pB.tile([128, 512], f32, tag="qs")
                            nc.scalar.copy(qs[:], ps[:])
                            qsw = pB.tile([128, 512], f32, tag="qsw")
                            nc.vector.stream_shuffle(qsw[:], qs[:], SHUF)
                            m2 = pB.tile([128, 512], f32, tag="m2")
                            nc.gpsimd.tensor_mul(m2[:], qsw[:], st_t[:])
                            qc = pB.tile([128, 512], f32, tag="qc")
                            nc.vector.tensor_mul(qc[:], ps[:], ct_t[:])
                            nc.vector.tensor_add(_sb(kT, sb), qc[:], m2[:])
                        # V projection + transpose to natural layout
                        wvt = pBw.tile([128, NHT, 128], f32r, tag="wB")
                        nc.sync.dma_start(
                            wvt[:], wv.rearrange("(o p) n -> p o n", p=128))
                        for sb in range(NSB):
                            ps = pBp.tile([128, 512], f32, tag="proj")
                            for ht in range(NHT):
                                nc.tensor.matmul(
                                    ps[:], wvt[:, ht, :],
                                    _sb(hT[:, ht, :], sb),
                                    start=(ht == 0), stop=(ht == NHT - 1))
                            vts = pB.tile([128, 512], f32, tag="vts")
                            nc.scalar.copy(vts[:], ps[:])
                            for k4 in range(4):
                                pt = pBp.tile([128, 128], f32, tag="vtr")
                                nc.tensor.transpose(
                                    pt[:], vts[:, k4 * 128:(k4 + 1) * 128],
                                    id_t[:])
                                nc.scalar.copy(v_nat[:, sb * 4 + k4, :],
                                               pt[:])

                    # ------- Phase C: per-head Q proj + rope + attention ----
                    if True:
                        with tc.tile_pool(name="pq", bufs=1) as pq, \
                             tc.tile_pool(name="pC", bufs=2) as pC, \
                             tc.tile_pool(name="pCw", bufs=1) as pCw, \
                             tc.tile_pool(name="pCp", bufs=2,
                                          space="PSUM") as pCp, \
                             tc.tile_pool(name="pCo", bufs=1,
                                          space="PSUM") as pCo:
                            for h in range(QH):
                                qTh = pq.tile([128, S], f32r, tag="qTh")
                                wqt = pCw.tile([128, NHT, 128], f32r,
                                               tag="wq")
                                nc.sync.dma_start(
                                    wqt[:],
                                    wq.rearrange("(o p) n -> p o n", p=128)
                                      [:, :, h * 128:(h + 1) * 128])
                                for sb in range(NSB):
                                    ps = pCp.tile([128, 512], f32,
                                                  tag="proj2")
                                    for ht in range(NHT):
                                        nc.tensor.matmul(
                                            ps[:], wqt[:, ht, :],
                                            _sb(hT[:, ht, :], sb),
                                            start=(ht == 0),
                                            stop=(ht == NHT - 1))
                                    ct_t = pC.tile([128, 512], f32,
                                                   tag="ropeC", bufs=1)
                                    st_t = pC.tile([128, 512], f32,
                                                   tag="ropeS", bufs=1)
                                    nc.sync.dma_start(ct_t[:], _sb(cq, sb))
                                    nc.sync.dma_start(st_t[:], _sb(s2q, sb))
                                    qs = pC.tile([128, 512], f32, tag="qs2", bufs=1)
                                    nc.scalar.copy(qs[:], ps[:])
                                    qsw = pC.tile([128, 512], f32,
                                                  tag="qsw2", bufs=1)
                                    nc.vector.stream_shuffle(qsw[:], qs[:],
                                                             SHUF)
                                    m2 = pC.tile([128, 512], f32, tag="m22", bufs=1)
                                    nc.gpsimd.tensor_mul(m2[:], qsw[:],
                                                         st_t[:])
                                    qc = pC.tile([128, 512], f32, tag="qc2", bufs=1)
                                    nc.vector.tensor_mul(qc[:], ps[:],
                                                         ct_t[:])
                                    nc.vector.tensor_add(_sb(qTh, sb),
                                                         qc[:], m2[:])
                                # attention for this head
                                for qb in range(NSB):
                                    acc = pCo.tile([128, 512], f32,
                                                   tag="acc")
                                    den = pCo.tile([1, 512], f32, tag="den")
                                    nkt = 4 * (qb + 1)
                                    for kt in range(nkt):
                                        j = kt - qb * 4
                                        coloff = max(0, j) * 128
                                        ncols = 512 - coloff
                                        qs0 = qb * 512 + coloff
                                        sc = pCp.tile([128, 512], f32,
                                                      tag="sc")
                                        nc.tensor.matmul(
                                            sc[:, 0:ncols],
                                            kT[:, kt * 128:(kt + 1) * 128],
                                            qTh[:, qs0:qs0 + ncols],
                                            start=True, stop=True)
                                        P = pC.tile([128, 512], f32r,
                                                    tag="P", bufs=3)
                                        nc.scalar.activation(
                                            P[:, 0:ncols], sc[:, 0:ncols],
                                            AF.Exp)
                                        if j >= 0:
                                            nc.vector.tensor_mul(
                                                P[:, 0:128], P[:, 0:128],
                                                tri_t[:])
                                        nc.tensor.matmul(
                                            acc[:, coloff:512],
                                            v_nat[:, kt, :], P[:, 0:ncols],
                                            start=(kt == 0),
                                            stop=(kt == nkt - 1))
                                        nc.tensor.matmul(
                                            den[0:1, coloff:512], ones_t[:],
                                            P[:, 0:ncols],
                                            start=(kt == 0),
                                            stop=(kt == nkt - 1))
                                    rd = pC.tile([1, 512], f32, tag="rd")
                                    nc.vector.reciprocal(rd[:], den[:])
                                    rb = pC.tile([128, 512], f32, tag="rb2")
                                    nc.gpsimd.partition_broadcast(rb[:],
                                                                  rd[:])
                                    ot = pC.tile([128, 512], f32r,
                                                 tag="ot")
                                    nc.vector.tensor_mul(ot[:], acc[:],
                                                         rb[:])
                                    nc.sync.dma_start(
                                        _sb(outd[h, :, :], qb), ot[:])

                        # ---- Phase D: Wo partial + AllReduce ----
                        with tc.tile_pool(name="pD", bufs=2) as pD, \
                             tc.tile_pool(name="pDw", bufs=1) as pDw, \
                             tc.tile_pool(name="pDp", bufs=2,
                                          space="PSUM") as pDp:
                            for ocg in range(2):
                                wo_g = pDw.tile([128, QH, 8, 128], f32r,
                                                tag="wo")
                                nc.sync.dma_start(
                                    wo_g[:],
                                    wo.rearrange("(a p) n -> p a n", p=128)
                                      .rearrange("p a (g n) -> p a g n",
                                                 n=128)
                                      [:, :, ocg * 8:(ocg + 1) * 8, :])
                                for sb in range(NSB):
                                    osb = pD.tile([128, QH, 512], f32r,
                                                  tag="osb")
                                    nc.sync.dma_start(
                                        osb[:],
                                        outd[:, :, sb * 512:(sb + 1) * 512]
                                        .rearrange("o p n -> p o n"))
                                    for oc8 in range(8):
                                        oc = ocg * 8 + oc8
                                        ps = pDp.tile([128, 512], f32,
                                                      tag="y")
                                        for k2 in range(QH):
                                            nc.tensor.matmul(
                                                ps[:],
                                                wo_g[:, k2, oc8, :],
                                                osb[:, k2, :],
                                                start=(k2 == 0),
                                                stop=(k2 == QH - 1))
                                        yt = pD.tile([128, 512], f32,
                                                     tag="yt")
                                        nc.scalar.copy(yt[:], ps[:])
                                        nc.sync.dma_start(
                                            _sb(ar_in[oc * 128:
                                                      (oc + 1) * 128,
                                                      :], sb), yt[:])
                            nc.gpsimd.collective_compute(
                                "AllReduce", AL.add, replica_groups=GROUPS,
                                ins=[ar_in.opt()], outs=[ar_out.opt()])

            # ---------- Phase E: x1 = xT + ar; rmsnorm2 -> h2T ----------
            with tc.tile_pool(name="ph2", bufs=1) as ph2:
                h2T = ph2.tile([128, NHT, S], f32r)
                with tc.tile_pool(name="pE", bufs=1) as pE, \
                     tc.tile_pool(name="pEs", bufs=2) as pEs, \
                     tc.tile_pool(name="pEp", bufs=2, space="PSUM") as pEp:
                    for sb in range(NSB):
                        x1sb = pE.tile([128, NHT, 512], f32, tag="x1sb")
                        ss_ps = pEp.tile([1, 512], f32, tag="ss2")
                        for ht in range(NHT):
                            xa = pEs.tile([128, 512], f32, tag="xa")
                            xb = pEs.tile([128, 512], f32, tag="xb")
                            nc.sync.dma_start(
                                xa[:],
                                _sb(xT[ht * 128:(ht + 1) * 128, :], sb))
                            nc.sync.dma_start(
                                xb[:],
                                _sb(ar_out[ht * 128:(ht + 1) * 128, :], sb))
                            nc.vector.tensor_add(x1sb[:, ht, :], xa[:],
                                                 xb[:])
                            nc.sync.dma_start(
                                _sb(x1d[ht * 128:(ht + 1) * 128, :], sb),
                                x1sb[:, ht, :])
                            sq = pEs.tile([128, 512], f32r, tag="sq2")
                            nc.scalar.activation(sq[:], x1sb[:, ht, :],
                                                 AF.Square)
                            nc.tensor.matmul(ss_ps[:], ones_t[:], sq[:],
                                             start=(ht == 0),
                                             stop=(ht == NHT - 1))
                        sd = pEs.tile([1, 512], f32, tag="sd2")
                        nc.scalar.activation(sd[:], ss_ps[:], AF.Sqrt,
                                             bias=eps_t[0:1, :],
                                             scale=1.0 / H)
                        rr = pEs.tile([1, 512], f32, tag="rr2")
                        nc.vector.reciprocal(rr[:], sd[:])
                        rb = pEs.tile([128, 512], f32, tag="rb3")
                        nc.gpsimd.partition_broadcast(rb[:], rr[:])
                        for ht in range(NHT):
                            nc.vector.scalar_tensor_tensor(
                                out=_sb(h2T[:, ht, :], sb),
                                in0=x1sb[:, ht, :],
                                scalar=wn2_t[:, ht:ht + 1],
                                in1=rb[:], op0=AL.mult, op1=AL.mult)

                # ---------- Phase F1: gate/up/silu-mul -> mT (DRAM) -------
                with tc.tile_pool(name="pF", bufs=2) as pF, \
                     tc.tile_pool(name="pFw", bufs=2) as pFw, \
                     tc.tile_pool(name="pFp", bufs=2, space="PSUM") as pFp:
                    for ct in range(FCT):
                        wgt = pFw.tile([128, NHT, 128], f32r, tag="wg")
                        wut = pFw.tile([128, NHT, 128], f32r, tag="wu")
                        nc.sync.dma_start(
                            wgt[:], wg.rearrange("(o p) n -> p o n", p=128)
                                      [:, :, ct * 128:(ct + 1) * 128])
                        nc.sync.dma_start(
                            wut[:], wu.rearrange("(o p) n -> p o n", p=128)
                                      [:, :, ct * 128:(ct + 1) * 128])
                        for sb in range(NSB):
                            pg = pFp.tile([128, 512], f32, tag="pg")
                            pu = pFp.tile([128, 512], f32, tag="pu")
                            for ht in range(NHT):
                                nc.tensor.matmul(
                                    pg[:], wgt[:, ht, :],
                                    _sb(h2T[:, ht, :], sb),
                                    start=(ht == 0), stop=(ht == NHT - 1))
                            for ht in range(NHT):
                                nc.tensor.matmul(
                                    pu[:], wut[:, ht, :],
                                    _sb(h2T[:, ht, :], sb),
                                    start=(ht == 0), stop=(ht == NHT - 1))
                            sg = pF.tile([128, 512], f32, tag="sg")
                            nc.scalar.activation(sg[:], pg[:], AF.Silu)
                            mt = pF.tile([128, 512], f32r, tag="mt")
                            nc.vector.tensor_mul(mt[:], pu[:], sg[:])
                            nc.sync.dma_start(
                                _sb(mTd[ct, :, :], sb), mt[:])

            # ---------- Phase F2: down + 0.25*x1 -> ReduceScatter --------
            with tc.tile_pool(name="pwd", bufs=1) as pwd, \
                 tc.tile_pool(name="pGm", bufs=2) as pGm, \
                 tc.tile_pool(name="pG", bufs=2) as pG, \
                 tc.tile_pool(name="pGp", bufs=2, space="PSUM") as pGp:
                wdt = pwd.tile([128, FCT, NHT, 128], f32r)
                for ct in range(FCT):
                    nc.sync.dma_start(
                        wdt[:, ct, :, :].rearrange("p a b -> p (a b)"),
                        wd[ct * 128:(ct + 1) * 128, :])
                for sb in range(NSB):
                    mm = pGm.tile([128, FCT, 512], f32r, tag="mm")
                    nc.sync.dma_start(
                        mm[:], mTd[:, :, sb * 512:(sb + 1) * 512]
                            .rearrange("o p n -> p o n"))
                    for oc in range(NHT):
                        ps = pGp.tile([128, 512], f32, tag="pd")
                        for ct in range(FCT):
                            nc.tensor.matmul(ps[:], wdt[:, ct, oc, :],
                                             mm[:, ct, :],
                                             start=(ct == 0),
                                             stop=(ct == FCT - 1))
                        x1t = pG.tile([128, 512], f32, tag="x1t")
                        nc.sync.dma_start(
                            x1t[:],
                            _sb(x1d[oc * 128:(oc + 1) * 128, :], sb))
                        yd = pG.tile([128, 512], f32, tag="yd")
                        nc.vector.scalar_tensor_tensor(
                            out=yd[:], in0=x1t[:], scalar=0.25,
                            in1=ps[:], op0=AL.mult, op1=AL.add)
                        nc.sync.dma_start(
                            rs_in[sb, oc * 128:(oc + 1) * 128, :], yd[:])
                nc.gpsimd.collective_compute(
                    "ReduceScatter", AL.add, replica_groups=GROUPS,
                    ins=[rs_in.opt()], outs=[rs_out.opt()])

            # ---------- Phase G: write output ----------
            nc.sync.dma_start(outsl[:], rs_out[:])

    nc.finalize()
    return nc


_CACHE = {}


def _get_nc():
    if "nc" not in _CACHE:
        _CACHE["nc"] = build()
    return _CACHE["nc"]


def _host_prep(inputs):
    """Build the 8 per-core input maps from the full problem inputs."""
    x = np.asarray(inputs["x"], np.float32)
    Wq = np.asarray(inputs["Wq"], np.float32)
    Wk = np.asarray(inputs["Wk"], np.float32)
    Wv = np.asarray(inputs["Wv"], np.float32)
    Wo = np.asarray(inputs["Wo"], np.float32)
    Wg = np.asarray(inputs["Wgate"], np.float32)
    Wu = np.asarray(inputs["Wup"], np.float32)
    Wd = np.asarray(inputs["Wdown"], np.float32)
    wn1v = np.asarray(inputs["w_norm1"], np.float32)
    wn2v = np.asarray(inputs["w_norm2"], np.float32)
    cos = np.asarray(inputs["freqs_cos"], np.float32)
    sin = np.asarray(inputs["freqs_sin"], np.float32)

    scale = 1.0 / float(np.sqrt(np.float32(HD)))
    Cq, S2q = make_rope_tables(cos, sin, scale)
    Ck, S2k = make_rope_tables(cos, sin, 1.0)
    tri_np = (np.arange(128)[None, :] >= np.arange(128)[:, None])
    tri_np = tri_np.astype(np.float32)
    wn1_np = np.ascontiguousarray(wn1v.reshape(NHT, 128).T)
    wn2_np = np.ascontiguousarray(wn2v.reshape(NHT, 128).T)
    ones_np = np.ones((128, 1), np.float32)
    id_np = np.eye(128, dtype=np.float32)

    shared = dict(cq=Cq, s2q=S2q, ck=Ck, s2k=S2k, wn1=wn1_np, wn2=wn2_np,
                  tri=tri_np, ones=ones_np, ident=id_np,
                  epsb=np.full((128, 1), EPS, np.float32))

    per_tp = []
    for tp in range(TPN):
        qcols = []
        for h in range(tp * QH, (tp + 1) * QH):
            qcols.extend(h * HD + PERM)
        per_tp.append(dict(
            wq=round_fp32r(Wq[:, qcols]),
            wk=round_fp32r(Wk[:, tp * HD + PERM]),
            wv=round_fp32r(np.ascontiguousarray(
                Wv[:, tp * HD:(tp + 1) * HD])),
            wo=round_fp32r(np.ascontiguousarray(
                Wo[tp * QH * HD:(tp + 1) * QH * HD, :])),
            wg=round_fp32r(np.ascontiguousarray(
                Wg[:, tp * FFS:(tp + 1) * FFS])),
            wu=round_fp32r(np.ascontiguousarray(
                Wu[:, tp * FFS:(tp + 1) * FFS])),
            wd=round_fp32r(np.ascontiguousarray(
                Wd[tp * FFS:(tp + 1) * FFS, :])),
        ))

    xTb = [np.ascontiguousarray(x[dp].T) for dp in range(2)]
    in_maps = []
    for c in range(8):
        dp, tp = c // 4, c % 4
        m = dict(shared)
        m.update(per_tp[tp])
        m["xT"] = xTb[dp]
        in_maps.append(m)
    return in_maps


def kernel(**inputs) -> np.ndarray:
    nc = _get_nc()
    in_maps = _host_prep(inputs)
    res = run_bass_kernel_spmd(nc, in_maps, core_ids=list(range(8)),
                               trace=False)
    out = np.zeros((B, S, H), np.float32)
    for c in range(8):
        dp, tp = c // 4, c % 4
        sl = res.results[c]["outsl"]          # [H, 512]
        out[dp, tp * SSL:(tp + 1) * SSL, :] = sl.T
    return out
